# revision 1
# baseline (speedup 1.0000x reference)
"""Trainium2 Bass kernel for nn_CausalFeatureTransformer.

Only the last sequence position (label token) of the reference output is needed,
so the per-sample transformer collapses:

  X_norm[n,f,:] = s[n,f]*u[f,:]*g1 + beta1  (f<256),  X_norm[n,256,:] = ln_a (const)
  u = feat_emb - rowmean(feat_emb),  s[n,f] = zn/sqrt(zn^2*rowvar(feat_emb)[f]+eps)

K/V rows become s[n,k]*UK[k,:]+const with UK = u@(diag(g1)Wk) shared across samples;
Q is one constant row; label-query attention = per-head elementwise score maps +
a [257]x[257,32] weighted sum per head done as PE matmuls. Scores/softmax run in
TRANSPOSED [k, n] layout so the per-head score map is a dual-scalar tensor_scalar
(a'[k], mask[k] are per-partition columns) and the attention matmul needs no
transposes of data-dependent tiles. Softmax denominators via PE ones-matvec
(exp args are O(10) for this data scale; no max-shift needed in fp32).

Data-parallel over N: 1024 samples -> 8 cores x 128 samples (full partition dim).
"""
import numpy as np
from contextlib import ExitStack

import concourse.bass as bass
import concourse.tile as tile
from concourse import bacc, mybir
from concourse.bass_utils import run_bass_kernel_spmd
from concourse.masks import make_identity

F32 = mybir.dt.float32
AF = mybir.ActivationFunctionType
OP = mybir.AluOpType
AX = mybir.AxisListType

N, FD, E, H, DK, SEQ = 1024, 256, 128, 4, 32, 257
NCORES = 8
NP = N // NCORES
EPS = 1e-5
ISQ = float(1.0 / np.sqrt(DK))
LOG1P9 = float(np.log1p(1e-9))

WCOL = {"wq": 0, "wk": 128, "wv": 256, "wo": 384, "w1": 512, "w2a": 768,
        "w2b": 896, "fe0": 1024, "fe1": 1152}
WPACK_W = 1280
VCOL = {"labT": 0, "bq": 1, "bv": 2, "bo": 3, "b1a": 4, "b1b": 5, "b2": 6,
        "g1": 7, "beta1": 8, "g2": 9, "beta2": 10, "alpha": 11}
VPACK_W = 12


def _body(tc, d, out_ap):
    nc = tc.nc
    ctx = ExitStack()
    with ctx:
        cp = ctx.enter_context(tc.tile_pool(name="cp", bufs=1))
        wp = ctx.enter_context(tc.tile_pool(name="wp", bufs=1))
        ps_m = ctx.enter_context(tc.tile_pool(name="ps_m", bufs=2, space="PSUM"))
        ps_a = ctx.enter_context(tc.tile_pool(name="ps_a", bufs=2, space="PSUM"))
        ps_o = ctx.enter_context(tc.tile_pool(name="ps_o", bufs=2, space="PSUM"))
        ps_s = ctx.enter_context(tc.tile_pool(name="ps_s", bufs=2, space="PSUM"))
        ps_t = ps_s

        def sb(name, shape, pool=cp):
            return pool.tile(list(shape), F32, tag=name, name=name)

        # ---------------- loads (batched) ----------------
        wpk = sb("wpk", [128, WPACK_W])
        nc.sync.dma_start(wpk[:], d["wpack"])
        vp = sb("vp", [128, VPACK_W])
        nc.sync.dma_start(vp[:], d["vpack"])
        apk = sb("apk", [128, 2 * SEQ])
        nc.sync.dma_start(apk[:], d["apack"])
        ar2 = sb("ar2", [1, SEQ])
        nc.sync.dma_start(ar2[:], d["arow2"])
        zt = sb("zt", [NP, FD])
        nc.sync.dma_start(zt[:], d["Z"])

        def W(name, w=128):
            return wpk[:, WCOL[name]:WCOL[name] + w]

        def V(name):
            return vp[:, VCOL[name]:VCOL[name] + 1]

        ar0, ar1 = apk[:, 0:SEQ], apk[:, SEQ:2 * SEQ]
        labT, g1c, beta1c = V("labT"), V("g1"), V("beta1")
        g2c, beta2c, b2c = V("g2"), V("beta2"), V("b2")

        epsT = sb("epsT", [128, 1]); nc.vector.memset(epsT[:], EPS)
        ident = sb("ident", [128, 128])
        make_identity(nc, ident[:])
        ones1 = sb("ones1", [1, 128]); nc.vector.memset(ones1[:], 1.0)
        onescol = sb("onescol", [128, 1]); nc.vector.memset(onescol[:], 1.0)

        # ---------------- c_max and mask columns ----------------
        m0 = sb("m0", [128, 1], wp)
        nc.vector.tensor_reduce(out=m0[:], in_=ar0, op=OP.max, axis=AX.X,
                                apply_absolute_value=True)
        m1 = sb("m1", [128, 1], wp)
        nc.vector.tensor_reduce(out=m1[:], in_=ar1, op=OP.max, axis=AX.X,
                                apply_absolute_value=True)
        m2 = sb("m2", [1, 1], wp)
        nc.vector.tensor_reduce(out=m2[:], in_=ar2[:], op=OP.max, axis=AX.X,
                                apply_absolute_value=True)
        mm = sb("mm", [128, 1], wp)
        nc.vector.tensor_tensor(out=mm[:], in0=m0[:], in1=m1[:], op=OP.max)
        p_mr = ps_t.tile([1, 128], F32, tag="sm")
        nc.tensor.transpose(p_mr[:], mm[:], ident[:])
        mrow = sb("mrow", [1, 128], wp)
        nc.scalar.copy(mrow[:], p_mr[:])
        mc = sb("mc", [1, 1], wp)
        nc.vector.tensor_reduce(out=mc[:], in_=mrow[:], op=OP.max, axis=AX.X)
        cmax = sb("cmax", [1, 1], wp)
        nc.vector.tensor_tensor(out=cmax[:], in0=mc[:], in1=m2[:], op=OP.max)
        rec = sb("rec", [1, 1], wp); nc.vector.reciprocal(rec[:], cmax[:])
        ge = sb("ge", [1, 1], wp)
        nc.vector.tensor_scalar(out=ge[:], in0=cmax[:], scalar1=1e-6, scalar2=None,
                                op0=OP.is_gt)
        recm1 = sb("recm1", [1, 1], wp)
        nc.vector.tensor_scalar_add(out=recm1[:], in0=rec[:], scalar1=-1.0)
        fsc = sb("fsc", [1, 1], wp)
        nc.vector.tensor_tensor(out=fsc[:], in0=ge[:], in1=recm1[:], op=OP.mult)
        nc.vector.tensor_scalar_add(out=fsc[:], in0=fsc[:], scalar1=1.0)
        gof = sb("gof", [1, 1], wp)
        nc.vector.tensor_scalar(out=gof[:], in0=ge[:], scalar1=-1e-3,
                                scalar2=1e-3 + 1e-9, op0=OP.mult, op1=OP.add)
        fcol = sb("fcol", [128, 1])
        nc.gpsimd.partition_broadcast(fcol[:], fsc[:])
        gcol = sb("gcol", [128, 1])
        nc.gpsimd.partition_broadcast(gcol[:], gof[:])
        # mask columns: ln(f*|A[k,256]| + g + 1e-9), k-chunks on partitions
        mkc = []
        for i, ar in enumerate((ar0, ar1)):
            ac = sb(f"ac{i}", [128, 1], wp)
            nc.vector.tensor_scalar(out=ac[:].bitcast(mybir.dt.int32),
                                    in0=ar[:, 256:257].bitcast(mybir.dt.int32),
                                    scalar1=0x7FFFFFFF, scalar2=None,
                                    op0=OP.bitwise_and)
            mk = sb(f"mk{i}", [128, 1])
            nc.scalar.activation(mk[:], ac[:], AF.Ln, bias=gcol[:, 0:1],
                                 scale=fcol[:, 0:1])
            mkc.append(mk)

        # ---------------- feat_emb stats: u, uT, varcol ----------------
        uts, vcols = [], []
        for i in range(2):
            fe = W("fe0") if i == 0 else W("fe1")
            st = sb(f"st{i}", [128, 6], wp)
            nc.vector.bn_stats(st[:], fe)
            ag = sb(f"ag{i}", [128, 2])
            nc.vector.bn_aggr(ag[:], st[:])
            u = sb(f"u{i}", [128, E])
            nc.vector.tensor_scalar(out=u[:], in0=fe, scalar1=ag[:, 0:1],
                                    scalar2=None, op0=OP.subtract)
            p_ut = ps_m.tile([128, 128], F32, tag="mm")
            nc.tensor.transpose(p_ut[:], u[:], ident[:])
            ut = sb(f"ut{i}", [128, 128])
            if i == 0:
                nc.scalar.copy(ut[:], p_ut[:])
            else:
                nc.vector.tensor_copy(out=ut[:], in_=p_ut[:])
            uts.append(ut)
            vcols.append(ag[:, 1:2])

        # ---------------- label-token norm (constant) ----------------
        p_ls = ps_s.tile([1, 1], F32, tag="sm")
        nc.tensor.matmul(p_ls[:], labT, onescol[:], start=True, stop=True)
        p_ls2 = ps_s.tile([1, 1], F32, tag="sm")
        nc.tensor.matmul(p_ls2[:], labT, labT, start=True, stop=True)
        mnL = sb("mnL", [1, 1], wp)
        nc.scalar.activation(mnL[:], p_ls[:], AF.Copy, bias=0.0, scale=1.0 / E)
        msqL = sb("msqL", [1, 1], wp)
        nc.vector.tensor_tensor(out=msqL[:], in0=mnL[:], in1=mnL[:], op=OP.mult)
        varL = sb("varL", [1, 1], wp)
        nc.vector.tensor_scalar(out=varL[:], in0=p_ls2[:], scalar1=1.0 / E,
                                scalar2=msqL[:, 0:1], op0=OP.mult, op1=OP.subtract)
        lvL = sb("lvL", [1, 1], wp)
        nc.scalar.activation(lvL[:], varL[:], AF.Ln, bias=epsT[0:1, :])
        rstdL = sb("rstdL", [1, 1], wp)
        nc.scalar.activation(rstdL[:], lvL[:], AF.Exp, scale=-0.5)
        mcol = sb("mcol", [128, 1])
        nc.gpsimd.partition_broadcast(mcol[:], mnL[:])
        rcol = sb("rcol", [128, 1])
        nc.gpsimd.partition_broadcast(rcol[:], rstdL[:])
        xl0 = sb("xl0", [E, 1], wp)
        nc.vector.tensor_scalar(out=xl0[:], in0=labT, scalar1=mcol[:, 0:1],
                                scalar2=rcol[:, 0:1], op0=OP.subtract, op1=OP.mult)
        dcol = sb("dcol", [E, 1])
        nc.vector.tensor_tensor(out=dcol[:], in0=xl0[:], in1=g1c, op=OP.mult)
        xlastT = sb("xlastT", [E, 1])
        nc.vector.tensor_tensor(out=xlastT[:], in0=dcol[:], in1=beta1c, op=OP.add)

        # ---------------- scaled weights ----------------
        wkp = sb("wkp", [E, E])
        nc.vector.tensor_scalar(out=wkp[:], in0=W("wk"), scalar1=g1c,
                                scalar2=None, op0=OP.mult)
        wvp = sb("wvp", [E, E])
        nc.vector.tensor_scalar(out=wvp[:], in0=W("wv"), scalar1=g1c,
                                scalar2=None, op0=OP.mult)
        w1p = sb("w1p", [E, 2 * E])
        nc.vector.tensor_scalar(out=w1p[:], in0=W("w1", 256), scalar1=g2c,
                                scalar2=None, op0=OP.mult)

        # ---------------- q row (constant over samples) ----------------
        p_q = ps_s.tile([128, 1], F32, tag="sm")
        nc.tensor.matmul(p_q[:], W("wq"), xlastT[:], start=True, stop=True)
        qcol = sb("qcol", [E, 1])
        nc.vector.tensor_scalar_add(out=qcol[:], in0=p_q[:], scalar1=V("bq"))
        # bo4[h, e] = 1 iff e//32 == h ; headmask = bo4.T ; qm = headmask*q
        bo4 = sb("bo4", [H, 128])
        nc.gpsimd.memset(bo4[:], 0.0)
        nc.gpsimd.affine_select(
            out=bo4[:].rearrange("p (g i) -> p g i", g=H),
            in_=bo4[:].rearrange("p (g i) -> p g i", g=H),
            compare_op=OP.not_equal, fill=1.0, base=0,
            pattern=[[-1, H], [0, 32]], channel_multiplier=1)
        p_hm = ps_s.tile([128, H], F32, tag="sm")
        nc.tensor.transpose(p_hm[:], bo4[:], ident[0:H, 0:H])
        headmask = sb("headmask", [E, H])
        nc.scalar.copy(headmask[:], p_hm[:])
        qm = sb("qm", [E, H])
        nc.vector.tensor_scalar(out=qm[:], in0=headmask[:], scalar1=qcol[:, 0:1],
                                scalar2=None, op0=OP.mult)

        # ---------------- a' columns: a[k,h] = (q_h . UK[k,hs])/sqrt(dk) --------
        p_wkt = ps_m.tile([128, 128], F32, tag="mm")
        nc.tensor.transpose(p_wkt[:], wkp[:], ident[:])
        wkpT = sb("wkpT", [E, E])
        nc.scalar.copy(wkpT[:], p_wkt[:])
        p_th = ps_s.tile([128, H], F32, tag="sm")
        nc.tensor.matmul(p_th[:], wkpT[:], qm[:], start=True, stop=True)
        th = sb("th", [E, H])
        nc.scalar.activation(th[:], p_th[:], AF.Copy, bias=0.0, scale=ISQ)
        acols = []
        for i in range(2):
            p_a = ps_s.tile([128, H], F32, tag="sm")
            nc.tensor.matmul(p_a[:], uts[i][:], th[:], start=True, stop=True)
            acol = sb(f"acol{i}", [128, H])
            nc.vector.tensor_copy(out=acol[:], in_=p_a[:])
            acols.append(acol)

        # ---------------- label-score consts: ecrow = exp(c''_h) ----------------
        p_kd = ps_s.tile([128, 1], F32, tag="sm")
        nc.tensor.matmul(p_kd[:], W("wk"), dcol[:], start=True, stop=True)
        kd = sb("kd", [E, 1], wp)
        nc.vector.tensor_copy(out=kd[:], in_=p_kd[:])
        prod = sb("prod", [E, 1], wp)
        nc.vector.tensor_tensor(out=prod[:], in0=qcol[:], in1=kd[:], op=OP.mult)
        p_c4 = ps_s.tile([H, 1], F32, tag="sm")
        nc.tensor.matmul(p_c4[:], headmask[:], prod[:], start=True, stop=True)
        c4 = sb("c4", [H, 1], wp)
        nc.scalar.activation(c4[:], p_c4[:], AF.Copy, bias=LOG1P9, scale=ISQ)
        p_cr = ps_s.tile([1, H], F32, tag="sm")
        nc.tensor.transpose(p_cr[:], c4[:], ident[0:H, 0:H])
        crow = sb("crow", [1, H], wp)
        nc.scalar.copy(crow[:], p_cr[:])
        ecrow = sb("ecrow", [1, H])
        nc.scalar.activation(ecrow[:], crow[:], AF.Exp)

        # ---------------- UV chunks + label V row ----------------
        uvs = []
        for i in range(2):
            p_uv = ps_m.tile([128, 128], F32, tag="mm")
            nc.tensor.matmul(p_uv[:], uts[i][:], wvp[:], start=True, stop=True)
            uv = sb(f"uv{i}", [128, E])
            if i == 0:
                nc.scalar.copy(uv[:], p_uv[:])
            else:
                nc.vector.tensor_copy(out=uv[:], in_=p_uv[:])
            uvs.append(uv)
        p_vd = ps_s.tile([128, 1], F32, tag="sm")
        nc.tensor.matmul(p_vd[:], W("wv"), dcol[:], start=True, stop=True)
        vdcol = sb("vdcol", [E, 1], wp)
        nc.vector.tensor_copy(out=vdcol[:], in_=p_vd[:])
        p_vdr = ps_t.tile([1, 128], F32, tag="sm")
        nc.tensor.transpose(p_vdr[:], vdcol[:], ident[:])
        vdrow = sb("vdrow", [1, E], wp)
        nc.scalar.copy(vdrow[:], p_vdr[:])
        # ulc[e] = vd[e]*exp(c''_{h(e)})  (label contribution, rank-1 over n)
        ulcrow = sb("ulcrow", [1, E])
        nc.vector.tensor_tensor(
            out=ulcrow[:].rearrange("p (g i) -> p g i", g=H),
            in0=vdrow[:].rearrange("p (g i) -> p g i", g=H),
            in1=ecrow[:].unsqueeze(2).broadcast_to((1, H, 32)), op=OP.mult)
        p_vc = ps_s.tile([128, 1], F32, tag="sm")
        nc.tensor.matmul(p_vc[:], W("wv"), beta1c, start=True, stop=True)
        vccol = sb("vccol", [E, 1])
        nc.vector.tensor_scalar_add(out=vccol[:], in0=p_vc[:], scalar1=V("bv"))

        # ---------------- FFN consts ----------------
        b1ps = []
        for i, bn in enumerate(("b1a", "b1b")):
            p_b1 = ps_s.tile([128, 1], F32, tag="sm")
            nc.tensor.matmul(p_b1[:], W("w1", 256)[:, 128 * i:128 * (i + 1)],
                             beta2c, start=True, stop=True)
            b1p = sb(f"b1p{i}", [128, 1])
            nc.vector.tensor_scalar_add(out=b1p[:], in0=p_b1[:], scalar1=V(bn))
            b1ps.append(b1p)
        alcol = sb("alcol", [E, 1])
        nc.gpsimd.partition_broadcast(alcol[:],
                                      vp[0:1, VCOL["alpha"]:VCOL["alpha"] + 1])
        cvec = sb("cvec", [E, 1])
        nc.vector.tensor_tensor(out=cvec[:], in0=alcol[:], in1=b2c, op=OP.mult)
        nc.vector.tensor_tensor(out=cvec[:], in0=cvec[:], in1=xlastT[:], op=OP.add)

        # ================= main phase =================
        stZ = sb("stZ", [NP, 6], wp); nc.vector.bn_stats(stZ[:], zt[:])
        agZ = sb("agZ", [NP, 2], wp); nc.vector.bn_aggr(agZ[:], stZ[:])
        # s = c/sqrt(c^2*v_f + eps*(varZ+eps)) with c = Z - mean: no Z-rstd needed
        zn = sb("zn", [NP, FD])
        nc.vector.tensor_scalar(out=zn[:], in0=zt[:], scalar1=agZ[:, 0:1],
                                scalar2=None, op0=OP.subtract)
        epsn = sb("epsn", [NP, 1], wp)
        nc.vector.tensor_scalar(out=epsn[:], in0=agZ[:, 1:2], scalar1=EPS,
                                scalar2=EPS * EPS, op0=OP.mult, op1=OP.add)
        p_en = ps_s.tile([1, 128], F32, tag="sm")
        nc.tensor.transpose(p_en[:], epsn[:], ident[:])
        enrow = sb("enrow", [1, 128], wp)
        nc.vector.tensor_copy(out=enrow[:], in_=p_en[:])
        epsnb = sb("epsnb", [128, 128])
        nc.gpsimd.partition_broadcast(epsnb[:], enrow[:])

        # transposed s, scores, softmax, weighted sums per k-chunk
        p_zA = ps_a.tile([128, 128], F32, tag="at")
        p_zB = ps_a.tile([128, 128], F32, tag="at")
        pz4 = ps_s.tile([128, H], F32, tag="sm")
        p_oA = ps_o.tile([64, 128], F32, tag="ao")
        p_oB = ps_o.tile([64, 128], F32, tag="ao")
        # initialize accumulators with the label-position rank-1 terms
        nc.tensor.matmul(pz4[:], ones1[:], ecrow[:], start=True, stop=False,
                         skip_group_check=True)
        nc.tensor.matmul(p_oA[:], ulcrow[:, 0:64], ones1[:], start=True, stop=False,
                         skip_group_check=True)
        nc.tensor.matmul(p_oB[:], ulcrow[:, 64:128], ones1[:], start=True,
                         stop=False, skip_group_check=True)
        for i, p_znT in enumerate((p_zA, p_zB)):
            nc.tensor.transpose(p_znT[:], zn[:, 128 * i:128 * (i + 1)], ident[:])
            sqT = wp.tile([128, 128], F32, tag=f"sqT{i}")
            nc.scalar.activation(sqT[:], p_znT[:], AF.Square)
            w1t = wp.tile([128, 128], F32, tag=f"w1t{i}")
            nc.vector.tensor_scalar(out=w1t[:], in0=sqT[:], scalar1=vcols[i],
                                    scalar2=None, op0=OP.mult)
            nc.vector.tensor_tensor(out=w1t[:], in0=w1t[:], in1=epsnb[:],
                                    op=OP.add)
            lnt = wp.tile([128, 128], F32, tag=f"lnt{i}")
            nc.scalar.activation(lnt[:], w1t[:], AF.Ln)
            rst = wp.tile([128, 128], F32, tag=f"rst{i}")
            nc.scalar.activation(rst[:], lnt[:], AF.Exp, scale=-0.5)
            sT = wp.tile([128, 128], F32, tag=f"sT{i}")
            nc.vector.tensor_tensor(out=sT[:], in0=p_znT[:], in1=rst[:], op=OP.mult)
            # scores [k, h, n] via dual-scalar ops
            scT = wp.tile([128, H, 128], F32, tag=f"scT{i}")
            for h in range(H):
                nc.vector.tensor_scalar(out=scT[:, h, :], in0=sT[:],
                                        scalar1=acols[i][:, h:h + 1],
                                        scalar2=mkc[i][:, 0:1],
                                        op0=OP.mult, op1=OP.add)
            eT = wp.tile([128, H, 128], F32, tag=f"eT{i}")
            nc.scalar.activation(eT[:], scT[:], AF.Exp)
            wpreT = wp.tile([128, H, 128], F32, tag=f"wpreT{i}")
            nc.vector.tensor_tensor(
                out=wpreT[:], in0=eT[:],
                in1=sT[:].unsqueeze(1).broadcast_to((128, H, 128)), op=OP.mult)
            for h in range(H):
                nc.tensor.matmul(pz4[:, h:h + 1], eT[:, h, :], onescol[:],
                                 start=False, stop=(i == 1 and h == H - 1),
                                 skip_group_check=True)
                p_o = p_oA if h < 2 else p_oB
                ls = slice(32 * (h % 2), 32 * (h % 2 + 1))
                nc.tensor.matmul(p_o[ls, :], uvs[i][:, 32 * h:32 * (h + 1)],
                                 wpreT[:, h, :], start=False,
                                 stop=(i == 1 and h >= 2), skip_group_check=True)
        # normalize: rzb[e, n] = 1/Z[h(e), n]
        rz4 = sb("rz4", [128, H], wp)
        nc.vector.reciprocal(rz4[:], pz4[:])
        p_rzT = ps_t.tile([H, 128], F32, tag="sm")
        nc.tensor.transpose(p_rzT[:], rz4[:], ident[:])
        rzT = sb("rzT", [H, 128], wp)
        nc.vector.tensor_copy(out=rzT[:], in_=p_rzT[:])
        p_rb = ps_m.tile([128, 128], F32, tag="mm")
        nc.tensor.matmul(p_rb[:], bo4[:], rzT[:], start=True, stop=True)
        rzb = sb("rzb", [128, 128], wp)
        nc.scalar.copy(rzb[:], p_rb[:])
        oaT = sb("oaT", [E, 128], wp)
        nc.vector.tensor_tensor(out=oaT[0:64, :], in0=p_oA[:], in1=rzb[0:64, :],
                                op=OP.mult)
        nc.vector.tensor_tensor(out=oaT[64:128, :], in0=p_oB[:],
                                in1=rzb[64:128, :], op=OP.mult)
        nc.vector.tensor_scalar_add(out=oaT[:], in0=oaT[:], scalar1=vccol[:, 0:1])

        # Wo + bo
        p_wo = ps_m.tile([128, 128], F32, tag="mm")
        nc.tensor.matmul(p_wo[:], W("wo"), oaT[:], start=True, stop=True)
        ooT = sb("ooT", [E, 128])
        nc.vector.tensor_scalar_add(out=ooT[:], in0=p_wo[:], scalar1=V("bo"))

        # LN over emb (stats need [n, e] layout)
        p_tn = ps_m.tile([128, 128], F32, tag="mm")
        nc.tensor.transpose(p_tn[:], ooT[:], ident[:])
        stO = sb("stO", [128, 6], wp); nc.vector.bn_stats(stO[:], p_tn[:])
        agO = sb("agO", [128, 2], wp); nc.vector.bn_aggr(agO[:], stO[:])
        vO = sb("vO", [128, 1], wp)
        nc.vector.tensor_scalar_add(out=vO[:], in0=agO[:, 1:2], scalar1=EPS)
        rstdO = sb("rstdO", [128, 1], wp)
        I32 = mybir.dt.int32
        nc.vector.tensor_scalar(out=rstdO[:].bitcast(I32), in0=vO[:].bitcast(I32),
                                scalar1=1, scalar2=None, op0=OP.arith_shift_right)
        nc.vector.tensor_scalar(out=rstdO[:].bitcast(I32), in0=rstdO[:].bitcast(I32),
                                scalar1=-1, scalar2=0x5F3759DF, op0=OP.mult,
                                op1=OP.add)
        nt = sb("nt", [128, 1], wp)
        for _ in range(3):
            nc.vector.tensor_tensor(out=nt[:], in0=rstdO[:], in1=rstdO[:],
                                    op=OP.mult)
            nc.vector.tensor_tensor(out=nt[:], in0=nt[:], in1=vO[:], op=OP.mult)
            nc.vector.tensor_scalar(out=nt[:], in0=nt[:], scalar1=-0.5,
                                    scalar2=1.5, op0=OP.mult, op1=OP.add)
            nc.vector.tensor_tensor(out=rstdO[:], in0=rstdO[:], in1=nt[:],
                                    op=OP.mult)
        hpre = sb("hpre", [128, 128], wp)
        nc.vector.tensor_scalar(out=hpre[:], in0=p_tn[:], scalar1=agO[:, 0:1],
                                scalar2=rstdO[:, 0:1], op0=OP.subtract, op1=OP.mult)
        p_ht = ps_m.tile([128, 128], F32, tag="mm")
        nc.tensor.transpose(p_ht[:], hpre[:], ident[:])
        hT = sb("hT", [128, 128], wp)
        nc.scalar.copy(hT[:], p_ht[:])

        # FFN
        gts = []
        for i in range(2):
            p_f1 = ps_m.tile([128, 128], F32, tag="mm")
            nc.tensor.matmul(p_f1[:], w1p[:, 128 * i:128 * (i + 1)], hT[:],
                             start=True, stop=True)
            gt = wp.tile([128, 128], F32, tag=f"gt{i}")
            nc.scalar.activation(gt[:], p_f1[:], AF.Gelu, bias=b1ps[i][:, 0:1])
            gts.append(gt)
        p_y = ps_m.tile([128, 128], F32, tag="mm")
        nc.tensor.matmul(p_y[:], W("w2a"), gts[0][:], start=True, stop=False)
        nc.tensor.matmul(p_y[:], W("w2b"), gts[1][:], start=False, stop=True)

        # final combine + transpose + store
        zf1 = sb("zf1", [128, 128], wp)
        nc.vector.tensor_tensor(out=zf1[:], in0=p_y[:], in1=ooT[:], op=OP.add)
        zfT = sb("zfT", [128, 128], wp)
        nc.vector.tensor_scalar(out=zfT[:], in0=zf1[:], scalar1=alcol[:, 0:1],
                                scalar2=cvec[:, 0:1], op0=OP.mult, op1=OP.add)
        p_zf = ps_m.tile([128, 128], F32, tag="mm")
        nc.tensor.transpose(p_zf[:], zfT[:], ident[:])
        zout = sb("zout", [128, 128], wp)
        nc.scalar.copy(zout[:], p_zf[:])
        nc.sync.dma_start(out_ap, zout[:])


_CACHE = {}


def _restrict_act_tables():
    """Limit the act-table-load pass to two sets so every non-Gelu activation
    (abs/copy/exp/identity/ln/square) resolves to one table and Gelu to the
    other -- avoids ~8 x 1.3us table reloads from per-function set churn."""
    import concourse.hw_specs as hws
    import concourse.bacc as bacc_mod
    orig = hws.get_activation_tables

    def patched(arch):
        t = orig(arch)
        keep = {}
        n_good = 0
        for name, fns in t.items():
            fnames = {f.name for f in fns}
            good = ("Ln" in fnames and "Exp" in fnames) or "Gelu" in fnames
            keep[name] = fns if good else set()   # keep positions for set ids
            n_good += bool(good)
        assert n_good >= 2, f"unexpected act table sets: {list(t)}"
        return keep

    bacc_mod.get_activation_tables = patched


def _get_nc():
    if "nc" in _CACHE:
        return _CACHE["nc"]
    _restrict_act_tables()
    nc = bacc.Bacc("TRN2", target_bir_lowering=False, debug=False,
                   num_devices=NCORES)
    d = {}
    for name, shape in (("wpack", (128, WPACK_W)), ("vpack", (128, VPACK_W)),
                        ("apack", (128, 2 * SEQ)), ("arow2", (1, SEQ)),
                        ("Z", (NP, FD))):
        d[name] = nc.dram_tensor(name, list(shape), F32, kind="ExternalInput").ap()
    out_ap = nc.dram_tensor("out", [NP, E], F32, kind="ExternalOutput").ap()
    with tile.TileContext(nc) as tc:
        _body(tc, d, out_ap)
    nc.compile()
    _CACHE["nc"] = nc
    return nc


def _in_maps(inputs):
    a = {k: np.ascontiguousarray(np.asarray(v, dtype=np.float32))
         for k, v in inputs.items()}
    wpack = np.zeros((128, WPACK_W), np.float32)
    wpack[:, 0:128] = a["Wq"]
    wpack[:, 128:256] = a["Wk"]
    wpack[:, 256:384] = a["Wv"]
    wpack[:, 384:512] = a["Wo"]
    wpack[:, 512:768] = a["W1"]
    wpack[:, 768:896] = a["W2"][0:128]
    wpack[:, 896:1024] = a["W2"][128:256]
    wpack[:, 1024:1152] = a["feat_emb"][0:128]
    wpack[:, 1152:1280] = a["feat_emb"][128:256]
    vpack = np.zeros((128, VPACK_W), np.float32)
    vpack[:, 0] = a["label_token"].reshape(E)
    for j, nm in ((1, "bq"), (2, "bv"), (3, "bo"), (6, "b2"), (7, "g1"),
                  (8, "beta1"), (9, "g2"), (10, "beta2")):
        vpack[:, j] = a[nm]
    vpack[:, 4] = a["b1"][0:128]
    vpack[:, 5] = a["b1"][128:256]
    vpack[0, 11] = float(np.asarray(a["alpha_res"]).reshape(-1)[0])
    apack = np.zeros((128, 2 * SEQ), np.float32)
    apack[:, 0:SEQ] = a["A_no_diag"][0:128]
    apack[:, SEQ:2 * SEQ] = a["A_no_diag"][128:256]
    arow2 = np.ascontiguousarray(a["A_no_diag"][256:257])
    shared = {"wpack": wpack, "vpack": vpack, "apack": apack, "arow2": arow2}
    maps = []
    for c in range(NCORES):
        m = dict(shared)
        m["Z"] = np.ascontiguousarray(a["Z"][c * NP:(c + 1) * NP])
        maps.append(m)
    return maps


def run(inputs, trace=False):
    nc = _get_nc()
    res = run_bass_kernel_spmd(nc, _in_maps(inputs), core_ids=list(range(NCORES)),
                               trace=trace)
    out = np.concatenate([res.results[c]["out"] for c in range(NCORES)], axis=0)
    return out.astype(np.float32), res


def kernel(**inputs):
    out, _ = run(inputs, trace=False)
    return out

